# revision 38
# baseline (speedup 1.0000x reference)
"""
LongConvolution (causal FFT conv) Trainium2 Bass kernel.

Problem: x (4, 8192, 1024) f32, filt (1024, 8192) f32.
  y[b, l, c] = sum_m x[b, m, c] * filt[c, l - m]   (causal, per-channel)

Strategy
--------
N = 16384 = 128*128 four-step FFT; every 128-point DFT stage is a tensor
engine matmul (fp16 operands, f32 PSUM accumulation).

Complex batch packing: convolution is linear in the signal, so batches
(b0,b1) pack into ONE complex signal z = b0 + i*b1. The per-channel
filter multiply commutes with the packing, so y0 = Re(out), y1 = Im(out)
with no unpack: this halves matmul and elementwise volume vs 4 real
signals.

Per complex signal (partition dim first; u indexes the 4 signals of a
2-channel group):
  m    [128, 128]  K-stacked [zre(n1=0:64); zim], l = 128*n1 + n2
  F1   A[n2, k1]   = m^T @ f1k (1 matmul, N=256 -> [Are|Aim])
  T1   DVE quad-mul only: tq = [A|A] * [TC|-TS|-TS|-TC]; the combine
       (Bre = q0-q1, Bim = q2-q3) is FOLDED into F2's PSUM accumulation
       via signed stationaries (8 matmuls N=512 per group).
  PW   DVE quad-mul against per-channel [Kre|Kim|Kim|-Kre]; single
       4D-strided DVE sub combines to P.
  I1   Q[k1, n2] = P^T @ (C + iS), data-stationary, 2 matmuls N=256.
  T2   DVE quad-mul only: wq = [Q|Q] * [TC|TS|TS|-TC]; combine folded
       into I2.
  I2   M-packed stationaries [gc|gs] etc. compute y_re in PSUM rows
       0:64 and y_im in rows 64:128 from the same rhs stream: 4 matmuls
       N=512 per group, single PSUM bank, single f16 copy-out.

All PSUM banks get exactly one start_tensor_calc (first matmul) and one
stop (last): start marks the bank's whole zero-region pending-zero, so
interleaving another region's start corrupts accumulation.

Engine balance per 2-channel group (64 groups/core): PE ~9216 cols,
DVE ~7168 fp16 elems (2x mode; gpsimd is avoided entirely - it steals
DVE SBUF ports), ACT ~3584 elems of PSUM->SBUF casts. DMAs are fused
(one x-load, one filter-load, one y-store per group; constants in a
single blob).

Sharding: d_model across 8 cores (128 channels each). Host pre/post:
x cast to fp16 and transposed per-core to (c, b, l); filter spectrum
precomputed on host (fp16, scrambled [k2,k1] layout, 1/N folded in);
output returns fp16 (c, b, l) -> f32 (b, l, c).
"""

import os
import sys

import numpy as np

for p in ("/opt/trn_rl_repo",):
    if p not in sys.path:
        sys.path.insert(0, p)

os.environ.setdefault("MYCRO_LOCAL_CACHE", "1")

# ----------------------------------------------------------------------------
# configuration
# ----------------------------------------------------------------------------
B, L, D = 4, 8192, 1024
NFFT = 2 * L               # 16384 = 128 * 128
NC = 8                     # cores
CPC = D // NC              # channels per core = 128
GC = 2                     # channels per group (DVE batching)
NG = CPC // GC             # 64 groups
U = GC * 2                 # signal slots per group (2 complex per channel)

# tuning knobs (build-time)
PW_ON_GPSIMD = os.environ.get("LC_PW_GPSIMD", "0") == "1"
FOLD_T1 = os.environ.get("LC_FOLD_T1", "0") == "1"   # fold T1 combine into F2
FOLD_T2 = os.environ.get("LC_FOLD_T2", "1") == "1"   # fold T2 combine into I2
DEBUG_DUMP = os.environ.get("LC_DEBUG", "0") == "1"  # dump group-0 intermediates
Y_ON_GPSIMD = os.environ.get("LC_Y_GPSIMD", "0") == "1"  # y PSUM->SBUF copy engine


def _consts():
    j = np.arange(128)
    ang128 = 2 * np.pi * np.outer(j, j) / 128
    angN = 2 * np.pi * np.outer(j, j) / NFFT
    C, S = np.cos(ang128), np.sin(ang128)
    TC, TS = np.cos(angN), np.sin(angN)
    C64, S64 = C[:64], S[:64]

    arrs = {}
    # F1 rhs: rows 0:64 (zre): [cos | -sin]; rows 64:128 (zim): [sin | cos]
    arrs["f1k"] = np.block([[C64, -S64], [S64, C64]])
    # T1 quad consts: q = [Are|Aim|Are|Aim] * [TC|-TS|-TS|-TC]
    #   Bre = q0 - q1 ; Bim = q2 - q3
    arrs["t1q"] = np.concatenate([TC, -TS, -TS, -TC], axis=1)
    # T2 quad consts: q = [Qre|Qim|Qre|Qim] * [TC|TS|TS|-TC]
    #   C're = q0 - q1 ; C'im = q2 - q3
    arrs["t2q"] = np.concatenate([TC, TS, TS, -TC], axis=1)
    # F2 stationaries
    arrs["f2c"] = C
    arrs["f2s"] = S
    arrs["f2sn"] = -S
    arrs["f2cn"] = -C
    arrs["f2cn"] = -C
    # I1 rhs: [-S | C | S]; rhs1 = [C|S] (cols 128:384), rhs2 = [-S|C] (0:256)
    arrs["i1m"] = np.concatenate([-S, C, S], axis=1)
    # I2 M-packed stationaries [k1, 128]: cols 0:64 -> y_re rows,
    # cols 64:128 -> y_im rows.
    gc, gs = C[:, :64], S[:, :64]
    arrs["w1"] = np.concatenate([gc, gs], axis=1)
    arrs["w1n"] = np.concatenate([-gc, -gs], axis=1)
    arrs["w2"] = np.concatenate([-gs, gc], axis=1)
    arrs["w2n"] = np.concatenate([gs, -gc], axis=1)
    return arrs


def _build_program():
    import concourse.bacc as bacc
    import concourse.mybir as mybir
    from concourse import tile

    f16 = mybir.dt.float16
    f32 = mybir.dt.float32

    nc = bacc.Bacc(None, target_bir_lowering=False, debug=False)

    # --- DRAM I/O ---
    xw = nc.dram_tensor("xw", (CPC, B, L), f16, kind="ExternalInput")
    kfq = nc.dram_tensor("kfq", (NG, 128, U, 512), f16, kind="ExternalInput")
    cblob_d = nc.dram_tensor("cblob", (128, CBLOB_W), f16, kind="ExternalInput")
    yw = nc.dram_tensor("yw", (CPC, B, L), f16, kind="ExternalOutput")
    dbg = {}
    if DEBUG_DUMP:
        for nm, shp in (
            ("dA", (128, U, 256)), ("dB", (128, U, 256)), ("dR", (128, U, 256)),
            ("dP", (128, U, 256)), ("dQ", (128, U, 256)), ("dW", (128, U, 512)),
        ):
            dbg[nm] = nc.dram_tensor(nm, shp, f16, kind="ExternalOutput")

    with tile.TileContext(nc) as tc:
        with (
            tc.tile_pool(name="const", bufs=1) as constp,
            tc.tile_pool(name="kf", bufs=3) as kfp,
            tc.tile_pool(name="m", bufs=3) as mp,
            tc.tile_pool(name="work", bufs=4) as wp,
            tc.tile_pool(name="out", bufs=3) as op,
            tc.tile_pool(name="psum", bufs=1, space="PSUM") as pp,
        ):
            # constants, one blob DMA
            cb = constp.tile([128, CBLOB_W], f16)
            nc.sync.dma_start(cb[:, 0:256], cblob_d[:, 0:256])
            nc.sync.dma_start(cb[:, 256:CBLOB_W], cblob_d[:, 256:CBLOB_W])
            _off = [0]

            def _cslice(w):
                a = _off[0]
                _off[0] += w
                return cb[:, a : a + w]

            f1k = _cslice(256)
            t1u = _cslice(U * 512).rearrange("p (u n) -> p u n", u=U)
            t2u = _cslice(U * 512).rearrange("p (u n) -> p u n", u=U)
            f2c = _cslice(128)
            f2s = _cslice(128)
            f2sn = _cslice(128)
            f2cn = _cslice(128)
            i1m = _cslice(384)
            w1 = _cslice(128)
            w1n = _cslice(128)
            w2 = _cslice(128)
            w2n = _cslice(128)

            for g in range(NG):
                kf = kfp.tile([128, U, 512], f16, tag="kf")
                nc.sync.dma_start(kf[:], kfq[g][:])

                m = mp.tile([128, U, 128], f16, tag="m")
                nc.sync.dma_start(
                    m[:].rearrange("p (c s) n -> p c s n", c=GC),
                    xw[GC * g : GC * g + GC].rearrange(
                        "c (s b) (a n) -> (b a) c s n", s=2, n=128
                    ),
                )

                # ---- F1: A[n2, k1] = m^T @ f1k per signal slot ----
                # pa: one 2-bank tile; one start/stop per bank.
                pa = pp.tile([128, U, 256], f32, tag="pa")
                for u in range(U):
                    nc.tensor.matmul(
                        pa[:, u, :], m[:, u, :], f1k[:],
                        start=(u % 2 == 0), stop=(u % 2 == 1),
                    )
                A_sb = wp.tile([128, U, 256], f16, tag="A")
                nc.scalar.copy(out=A_sb[:], in_=pa[:])
                if DEBUG_DUMP and g == 0:
                    nc.sync.dma_start(dbg["dA"][:], A_sb[:])

                # ---- T1 quads: tq = [A|A] * [TC|-TS|-TS|-TC] ----
                tq = wp.tile([128, U, 512], f16, tag="tq")
                nc.vector.tensor_mul(
                    tq[:].rearrange("p u (r n) -> p u r n", r=2),
                    A_sb[:].rearrange("p u (r n) -> p u r n", r=1)
                    .broadcast_to([128, U, 2, 256]),
                    t1u[:].rearrange("p u (r n) -> p u r n", r=2),
                )

                # ---- F2 (T1 combine folded into PSUM accumulation) ----
                # Bre = tq0 - tq1, Bim = tq2 - tq3
                # re = C@Bre + S@Bim ; im = C@Bim - S@Bre
                pr_re = pp.tile([128, U, 128], f32, tag="prre")
                pr_im = pp.tile([128, U, 128], f32, tag="prim")

                def tv(k):
                    return tq[:, :, 128 * k : 128 * (k + 1)]

                nc.tensor.matmul(pr_re[:], f2c[:], tv(0), start=True, stop=False)
                nc.tensor.matmul(pr_im[:], f2c[:], tv(2), start=True, stop=False)
                nc.tensor.matmul(pr_re[:], f2cn[:], tv(1), start=False, stop=False)
                nc.tensor.matmul(pr_im[:], f2cn[:], tv(3), start=False, stop=False)
                nc.tensor.matmul(pr_re[:], f2s[:], tv(2), start=False, stop=False)
                nc.tensor.matmul(pr_im[:], f2s[:], tv(1), start=False, stop=False)
                nc.tensor.matmul(pr_re[:], f2sn[:], tv(3), start=False, stop=True)
                nc.tensor.matmul(pr_im[:], f2sn[:], tv(0), start=False, stop=True)
                R_sb = wp.tile([128, U, 256], f16, tag="R")
                nc.scalar.copy(out=R_sb[:, :, 0:128], in_=pr_re[:])
                nc.scalar.copy(out=R_sb[:, :, 128:256], in_=pr_im[:])
                if DEBUG_DUMP and g == 0:
                    nc.sync.dma_start(dbg["dR"][:], R_sb[:])

                # ---- PW quads: pq = [R|R] * [Kre|Kim | Kim|Kre] ----
                pq = wp.tile([128, U, 512], f16, tag="pq")
                nc.vector.tensor_mul(
                    pq[:].rearrange("p u (r n) -> p u r n", r=2),
                    R_sb[:].rearrange("p u (r n) -> p u r n", r=1)
                    .broadcast_to([128, U, 2, 256]),
                    kf[:].rearrange("p u (r n) -> p u r n", r=2),
                )

                # ---- PW combines: Pre = q0 - q1 ; Pim = q2 + q3 ----
                P_sb = wp.tile([128, U, 256], f16, tag="P")
                pq_v = pq[:].rearrange("p u (a n) -> p u a n", a=4)
                nc.vector.tensor_sub(
                    P_sb[:].rearrange("p u (a n) -> p u a n", a=2),
                    pq_v[:, :, 0::2, :],
                    pq_v[:, :, 1::2, :],
                )
                if DEBUG_DUMP and g == 0:
                    nc.sync.dma_start(dbg["dP"][:], P_sb[:])

                # ---- I1: Q[k1, n2] = P^T @ (C + iS), data-stationary ----
                # pc: one 2-bank tile; one start per bank (first mm of u=0/u=2),
                # one stop per bank (last mm of u=1/u=3).
                pc = pp.tile([128, U, 256], f32, tag="pc")
                for u in range(U):
                    nc.tensor.matmul(
                        pc[:, u, :], P_sb[:, u, 0:128], i1m[:, 128:384],
                        start=(u % 2 == 0), stop=False,
                    )
                    nc.tensor.matmul(
                        pc[:, u, :], P_sb[:, u, 128:256], i1m[:, 0:256],
                        start=False, stop=(u % 2 == 1),
                    )
                Q_sb = wp.tile([128, U, 256], f16, tag="Q")
                nc.scalar.copy(out=Q_sb[:], in_=pc[:])
                if DEBUG_DUMP and g == 0:
                    nc.sync.dma_start(dbg["dQ"][:], Q_sb[:])

                # ---- T2 quads: wq = [Q|Q] * [TC|TS|TS|-TC] ----
                wq = wp.tile([128, U, 512], f16, tag="wq")
                nc.vector.tensor_mul(
                    wq[:].rearrange("p u (r n) -> p u r n", r=2),
                    Q_sb[:].rearrange("p u (r n) -> p u r n", r=1)
                    .broadcast_to([128, U, 2, 256]),
                    t2u[:].rearrange("p u (r n) -> p u r n", r=2),
                )
                if DEBUG_DUMP and g == 0:
                    nc.sync.dma_start(dbg["dW"][:], wq[:])

                # ---- I2 (M-packed, fused N=512): py rows 0:64 = y_re,
                # rows 64:128 = y_im; C're = w0-w1, C'im = w2-w3 folded via
                # signed stationaries.
                py = pp.tile([128, U, 128], f32, tag="py", bufs=2)

                def wv(k):
                    return wq[:, :, 128 * k : 128 * (k + 1)]

                nc.tensor.matmul(py[:], w1[:], wv(0), start=True, stop=False)
                nc.tensor.matmul(py[:], w1n[:], wv(1), start=False, stop=False)
                nc.tensor.matmul(py[:], w2[:], wv(2), start=False, stop=False)
                nc.tensor.matmul(py[:], w2n[:], wv(3), start=False, stop=True)

                ysb = op.tile([128, U, 128], f16, tag="ysb")
                nc.scalar.copy(out=ysb[:], in_=py[:])

                # ---- store: one DMA; src partition p = (b, a) ----
                nc.sync.dma_start(
                    yw[GC * g : GC * g + GC].rearrange(
                        "c (s b) (a n) -> (b a) c s n", s=2, n=128
                    ),
                    ysb[:].rearrange("p (c s) n -> p c s n", c=GC),
                )

    nc.compile()
    return nc


CONST_ORDER = (
    ("f1k", 256), ("t1u", U * 512), ("t2u", U * 512), ("f2c", 128),
    ("f2s", 128), ("f2sn", 128), ("f2cn", 128), ("i1m", 384),
    ("w1", 128), ("w1n", 128), ("w2", 128), ("w2n", 128),
)
CBLOB_W = sum(w for _, w in CONST_ORDER)


def _host_arrays():
    cst = _consts()
    arrs = {k: v.astype(np.float16) for k, v in cst.items()}
    # replicated twiddle quads (flat [128, U*512])
    arrs["t1u"] = np.ascontiguousarray(
        np.broadcast_to(arrs.pop("t1q")[:, None, :], (128, U, 512))
    ).reshape(128, U * 512)
    arrs["t2u"] = np.ascontiguousarray(
        np.broadcast_to(arrs.pop("t2q")[:, None, :], (128, U, 512))
    ).reshape(128, U * 512)
    blob = np.concatenate([arrs[k] for k, _ in CONST_ORDER], axis=1)
    assert blob.shape == (128, CBLOB_W), blob.shape
    return {"cblob": np.ascontiguousarray(blob)}


def _prep_inputs(x, filt):
    consts = _host_arrays()

    # filter spectrum: FFT of zero-padded filter; reshape(128,128) IS the
    # scrambled [k2, k1] layout of the on-device four-step forward.
    kpad = np.zeros((D, NFFT), np.float64)
    kpad[:, :L] = filt
    Kf = (np.fft.fft(kpad, axis=1) / NFFT).reshape(D, 128, 128)
    Kre = Kf.real.astype(np.float16)
    Kim = Kf.imag.astype(np.float16)
    # per-channel [128, 384] = [Kre | Kim | Kre]
    kq = np.concatenate([Kre, Kim, Kim, -Kre], axis=2)  # (D, 128, 512)

    x16 = x.astype(np.float16)
    in_maps = []
    for ci in range(NC):
        sl = slice(ci * CPC, (ci + 1) * CPC)
        m = dict(consts)
        m["xw"] = np.ascontiguousarray(x16[:, :, sl].transpose(2, 0, 1))
        # kfq[g, p, u, :] = kq[core_base + 2g + u//2, p, :]
        m["kfq"] = np.ascontiguousarray(
            np.repeat(
                kq[sl].reshape(NG, GC, 128, 512).transpose(0, 2, 1, 3), 2, axis=2
            )
        )
        in_maps.append(m)
    return in_maps


def kernel(x: np.ndarray, filt: np.ndarray) -> np.ndarray:
    from concourse.bass_utils import run_bass_kernel_spmd

    assert x.shape == (B, L, D) and filt.shape == (D, L)
    x = np.ascontiguousarray(x, dtype=np.float32)
    filt = np.ascontiguousarray(filt, dtype=np.float32)

    in_maps = _prep_inputs(x, filt)
    nc = _build_program()
    res = run_bass_kernel_spmd(nc, in_maps, core_ids=list(range(NC)))

    y = np.empty((B, L, D), np.float32)
    for ci in range(NC):
        sl = slice(ci * CPC, (ci + 1) * CPC)
        y[:, :, sl] = res.results[ci]["yw"].astype(np.float32).transpose(1, 2, 0)
    return y


def run_profiled(inputs):
    """Build + run with NTFF tracing; returns BassKernelResults (test-only)."""
    from concourse.bass_utils import run_bass_kernel_spmd

    x = np.ascontiguousarray(inputs["x"], dtype=np.float32)
    filt = np.ascontiguousarray(inputs["filt"], dtype=np.float32)
    in_maps = _prep_inputs(x, filt)
    nc = _build_program()
    return run_bass_kernel_spmd(
        nc, in_maps, core_ids=list(range(NC)), trace=True
    )


if __name__ == "__main__":
    rng = np.random.default_rng(0)
    x = rng.standard_normal((B, L, D)).astype(np.float32)
    filt = rng.standard_normal((D, L)).astype(np.float32)
    y = kernel(x, filt)
    print("y", y.shape, y.dtype, float(np.abs(y).max()))
